# revision 2
# baseline (speedup 1.0000x reference)
"""FPS (npoint=2) Bass kernel v5.

Per core (8 batches, planes [128,2048] fp32):
- DMA order staggers y early (centroid latency), x spread late (d-phase
  cadence). All HWDGE triggers issued first.
- y argmax: per-batch rowmax, partition_all_reduce finales (no PE
  transposes), ONE batched indirect row-gather per 4-batch group
  ([P,4,16]), tiny in-row localization, grouped centroid gather.
- d phase: Scalar squares (bias=-c), add1 on GpSimd, add2 split: cols
  [0,PSW) summed on PE into PSUM (identity fp32 matmuls), cols
  [PSW,2048) via DVE tensor_add in-place; MAX8+FIND per piece; one
  PAR-based finale at the end.
"""

import os

import numpy as np

import concourse.bacc as bacc
import concourse.bass as bass
import concourse.bass_isa as bass_isa
import concourse.mybir as mybir
from concourse.masks import make_identity
from concourse.tile import TileContext

B = 64
N_CORES = 8
BPC = B // N_CORES
N = 262144
P = 128
COLS = N // P  # 2048
GRP = 4

F32 = mybir.dt.float32
U32 = mybir.dt.uint32
I32 = mybir.dt.int32
AX = mybir.AxisListType.X
OP = mybir.AluOpType
RED = bass_isa.ReduceOp
SQUARE = mybir.ActivationFunctionType.Square

# tuning knobs
PSW = int(os.environ.get("V5_PSW", "1536"))     # psum (PE-add) width, 0|512|1024|1536
YSPLIT = int(os.environ.get("V5_YSPLIT", "8"))  # y-reds b < YSPLIT on DVE, rest GpSimd

SBW = COLS - PSW

DMA_ORDER = (
    [("y", b) for b in range(8)]
    + [p for b in range(8) for p in (("z", b), ("x", b))]
)
AS = int(os.environ.get("V5_AS", "512"))  # add1 cols [0:AS] on DVE, rest GpSimd


def build_nc():
    nc = bacc.Bacc()
    xin = nc.dram_tensor("xyz", [BPC, 3, N], F32, kind="ExternalInput")
    out = nc.dram_tensor("idx", [1, 2 * BPC], I32, kind="ExternalOutput")
    xrows16 = xin.rearrange("b c n -> (b c n)").rearrange("(r k) -> r k", k=16)
    xflat = xin.rearrange("b c n -> (b c n)")[:, None]

    with TileContext(nc) as tc:
        with (
            tc.tile_pool(name="consts", bufs=1) as consts,
            tc.tile_pool(name="yp", bufs=BPC) as yp,
            tc.tile_pool(name="zp", bufs=BPC) as zp,
            tc.tile_pool(name="xp", bufs=BPC) as xp,
            tc.tile_pool(name="sm", bufs=2) as sm,
            tc.tile_pool(name="acc", bufs=1) as acc,
            tc.tile_pool(name="pd", bufs=2, space="PSUM") as pdp,
            tc.tile_pool(name="psS", bufs=1, space="PSUM") as psS,
        ):
            # ---- plane DMAs first (Sync engine streams them in order) ----
            tys, tzs, txs = [None] * BPC, [None] * BPC, [None] * BPC
            pools = {"y": (yp, tys, 1), "z": (zp, tzs, 2), "x": (xp, txs, 0)}
            for kind, b in DMA_ORDER:
                pool, store, c = pools[kind]
                t = pool.tile([P, COLS], F32, tag="t")
                store[b] = t
                nc.sync.dma_start(t, xin[b, c].rearrange("(p m) -> p m", p=P))

            # ---- constants ----
            ident = consts.tile([P, P], F32)
            make_identity(nc, ident)
            ones = consts.tile([1, P], F32)
            nc.vector.memset(ones, 1.0)

            def iota_f32(tag, shape, pattern, base, chan):
                ti = consts.tile(shape, I32, tag=f"{tag}_i")
                nc.gpsimd.iota(ti, pattern=pattern, base=base, channel_multiplier=chan)
                tf = consts.tile(shape, F32, tag=f"{tag}_f")
                nc.vector.tensor_copy(tf, ti)
                return tf

            # revb[p] = N - 2048 p ; revb2[p] = N - PSW - 2048 p
            revb_f = iota_f32("revb", [P, 1], [[0, 1]], N, -COLS)
            revb2_f = (
                iota_f32("revb2", [P, 1], [[0, 1]], N - PSW, -COLS) if PSW else None
            )
            # cand16[p, j] = 2048 - (16 p + j)
            cand16 = iota_f32("cand16", [P, 16], [[-1, 16]], COLS, -16)
            # candP8[p, b] = 128 - p
            candP8 = iota_f32("candP8", [P, 8], [[0, 8]], P, -1)
            # colbase8[p, b] = p + (b*3N + N)/16 + 128*128
            # (iota step limit is int16, so build as 16*(b*3072 + 2048) + p)
            iotaP_f = iota_f32("iotaP", [P, 1], [[0, 1]], 0, 1)
            cA = iota_f32("cA", [P, 8], [[3 * N // 256, 8]], (N // 16 + P * P) // 16, 0)
            colbase8 = consts.tile([P, 8], F32, tag="colbase8")
            nc.vector.tensor_scalar(
                out=colbase8, in0=cA, scalar1=16.0, scalar2=iotaP_f,
                op0=OP.mult, op1=OP.add,
            )
            # bpn12[q] = N * q  (q = 3*(b-lo) + c)
            bpn12 = iota_f32("bpn12", [3 * GRP, 1], [[0, 1]], 0, N)
            # selmask[q, b] = 1 iff q//3 == b, for q in 0..11, b in 0..3:
            #   (q >= 3b) && (q <= 3b + 2)
            qv = iota_f32("qv", [3 * GRP, GRP], [[0, GRP]], 0, 1)
            c3 = iota_f32("c3", [3 * GRP, GRP], [[3, GRP]], 0, 0)
            m1 = consts.tile([3 * GRP, GRP], F32, tag="m1")
            nc.vector.tensor_tensor(m1, qv, c3, op=OP.is_ge)
            m2 = consts.tile([3 * GRP, GRP], F32, tag="m2")
            nc.vector.tensor_scalar(
                out=m2, in0=c3, scalar1=2.0, scalar2=None, op0=OP.add
            )
            selmask = consts.tile([3 * GRP, GRP], F32, tag="selmask")
            nc.vector.tensor_tensor(m2, qv, m2, op=OP.is_le)
            nc.vector.tensor_tensor(selmask, m1, m2, op=OP.mult)

            out_i = acc.tile([1, 2 * BPC], I32)
            dY = acc.tile([P, 8], F32)
            mxAll = acc.tile([P, 8], F32)
            wcAll = acc.tile([P, 8], F32)
            wr = acc.tile([P, 8], F32)
            wjAll = acc.tile([P, 8], F32)
            idxAll = acc.tile([P, 8], F32)
            offs8u = acc.tile([P, 8], U32)
            growA = acc.tile([P, 2 * GRP * 16], F32)
            negcY = acc.tile([P, 8], F32)
            negc_g = [None, None]
            dV8p = acc.tile([P, 8 * BPC], F32)
            dI8p = acc.tile([P, 8 * BPC], U32)
            dV8s = acc.tile([P, 8 * BPC], F32)
            dI8s = acc.tile([P, 8 * BPC], U32)

            def col0(t, k=8):
                return t.rearrange("p (b k) -> p b k", k=k)[:, :, 0]

            # ---------- y phase ----------
            def y_reduce(b):
                eng = nc.vector if b < YSPLIT else nc.gpsimd
                eng.tensor_reduce(dY[:, b : b + 1], tys[b], axis=AX, op=OP.max)

            def y_finale(g):
                lo = g * GRP
                sl = slice(lo, lo + GRP)
                nc.gpsimd.partition_all_reduce(
                    mxAll[:, sl], dY[:, sl], channels=P, reduce_op=RED.max
                )
                nc.vector.tensor_scalar(
                    out=negcY[:, sl], in0=mxAll[:, sl], scalar1=-1.0,
                    scalar2=None, op0=OP.mult,
                )
                eq = sm.tile([P, GRP], F32, tag=f"eq{g}")
                nc.vector.tensor_tensor(eq, dY[:, sl], mxAll[:, sl], op=OP.is_equal)
                nc.vector.tensor_tensor(eq, eq, candP8[:, 0:GRP], op=OP.mult)
                nc.gpsimd.partition_all_reduce(
                    wcAll[:, sl], eq, channels=P, reduce_op=RED.max
                )
                # row-gather offsets: colbase8[:, b] - 128*wc
                offs_f = sm.tile([P, GRP], F32, tag=f"offs{g}")
                for b in range(lo, lo + GRP):
                    nc.vector.tensor_scalar(
                        out=offs_f[:, b - lo : b - lo + 1],
                        in0=wcAll[:, b : b + 1],
                        scalar1=-float(P), scalar2=colbase8[:, b : b + 1],
                        op0=OP.mult, op1=OP.add,
                    )
                nc.vector.tensor_copy(offs8u[:, sl], offs_f)
                for b in range(lo, lo + GRP):
                    nc.gpsimd.indirect_dma_start(
                        out=growA[:, b * 16 : (b + 1) * 16], out_offset=None,
                        in_=xrows16,
                        in_offset=bass.IndirectOffsetOnAxis(
                            ap=offs8u[:, b : b + 1], axis=0
                        ),
                    )
                for b in range(lo, lo + GRP):
                    c2 = sm.tile([P, 16], F32, tag=f"c2{g}")
                    nc.vector.scalar_tensor_tensor(
                        out=c2, in0=growA[:, b * 16 : b * 16 + 16],
                        scalar=mxAll[:, b : b + 1], in1=cand16,
                        op0=OP.is_equal, op1=OP.mult,
                    )
                    nc.vector.tensor_reduce(wr[:, b : b + 1], c2, axis=AX, op=OP.max)
                nc.gpsimd.partition_all_reduce(
                    wjAll[:, sl], wr[:, sl], channels=P, reduce_op=RED.max
                )
                # idx = 264192 - 2048*wc - wj
                tmp = sm.tile([P, GRP], F32, tag=f"tmp{g}")
                nc.vector.tensor_scalar(
                    out=tmp, in0=wjAll[:, sl], scalar1=-1.0,
                    scalar2=float(COLS * (P + 1)), op0=OP.mult, op1=OP.add,
                )
                wt = sm.tile([P, GRP], F32, tag=f"wt{g}")
                nc.vector.tensor_scalar(
                    out=wt, in0=wcAll[:, sl], scalar1=-float(COLS), scalar2=None,
                    op0=OP.mult,
                )
                nc.vector.tensor_tensor(idxAll[:, sl], wt, tmp, op=OP.add)
                nc.scalar.copy(out_i[0:1, lo : lo + GRP], idxAll[0:1, sl])
                # centroid gather: sel12[q] = idxAll[q, lo + q//3]
                selv = sm.tile([3 * GRP, GRP], F32, tag=f"selv{g}")
                nc.vector.tensor_tensor(
                    selv, idxAll[0 : 3 * GRP, sl], selmask, op=OP.mult
                )
                sel12 = sm.tile([3 * GRP, 1], F32, tag=f"sel12{g}")
                nc.vector.tensor_reduce(sel12, selv, axis=AX, op=OP.max)
                offs12f = sm.tile([3 * GRP, 1], F32, tag=f"offs12f{g}")
                nc.vector.tensor_scalar(
                    out=offs12f, in0=sel12, scalar1=float(lo * 3 * N),
                    scalar2=bpn12, op0=OP.add, op1=OP.add,
                )
                offs12 = sm.tile([3 * GRP, 1], U32, tag=f"offs12u{g}")
                nc.vector.tensor_copy(offs12, offs12f)
                cg = sm.tile([3 * GRP, 1], F32, tag=f"cg{g}")
                nc.gpsimd.indirect_dma_start(
                    out=cg, out_offset=None, in_=xflat,
                    in_offset=bass.IndirectOffsetOnAxis(ap=offs12, axis=0),
                )
                pcr = psS.tile([1, 3 * GRP], F32, tag="pcr")
                nc.tensor.transpose(pcr, cg, ident[0 : 3 * GRP, 0 : 3 * GRP])
                negrow = sm.tile([1, 3 * GRP], F32, tag=f"negrow{g}")
                nc.scalar.mul(negrow, pcr, -1.0)
                pneg = psS.tile([P, 3 * GRP], F32, tag="pneg")
                nc.tensor.matmul(pneg, ones, negrow, start=True, stop=True)
                negc = sm.tile([P, 3 * GRP], F32, tag=f"negc{g}")
                nc.scalar.copy(negc, pneg)
                negc_g[g] = negc

            def nbias(b, c):
                if c == 1:
                    return negcY[:, b : b + 1]
                g = b // GRP
                j = 3 * (b - GRP * g) + c
                return negc_g[g][:, j : j + 1]

            # ---------- d phase (split into emission stages) ----------
            def d_fronty(b):
                nc.scalar.activation(tys[b], tys[b], SQUARE, bias=nbias(b, 1))

            def d_frontz(b):
                nc.scalar.activation(tzs[b], tzs[b], SQUARE, bias=nbias(b, 2))

            def d_add1(b):
                ty, tz = tys[b], tzs[b]
                if AS:
                    nc.vector.tensor_add(tz[:, 0:AS], ty[:, 0:AS], tz[:, 0:AS])
                if AS < COLS:
                    nc.gpsimd.tensor_add(tz[:, AS:], ty[:, AS:], tz[:, AS:])

            def d_sqx(b):
                nc.scalar.activation(txs[b], txs[b], SQUARE, bias=nbias(b, 0))

            def d_tail(b):
                tz, tx = tzs[b], txs[b]
                if PSW:
                    pd = pdp.tile([P, PSW], F32, tag="pd")
                    for ch in range(PSW // 512):
                        slc = slice(512 * ch, 512 * (ch + 1))
                        nc.tensor.matmul(pd[:, slc], ident, tz[:, slc],
                                         start=True, stop=False)
                        nc.tensor.matmul(pd[:, slc], ident, tx[:, slc],
                                         start=False, stop=True)
                sb = slice(PSW, COLS)
                nc.gpsimd.tensor_add(tx[:, sb], tz[:, sb], tx[:, sb])
                nc.vector.max(out=dV8s[:, 8 * b : 8 * b + 8], in_=tx[:, sb])
                nc.vector.max_index(dI8s[:, 8 * b : 8 * b + 8],
                                    dV8s[:, 8 * b : 8 * b + 8], tx[:, sb])
                if PSW:
                    nc.vector.max(out=dV8p[:, 8 * b : 8 * b + 8], in_=pd)
                    nc.vector.max_index(dI8p[:, 8 * b : 8 * b + 8],
                                        dV8p[:, 8 * b : 8 * b + 8], pd)

            def d_finale():
                vs = col0(dV8s)
                cs = sm.tile([P, 8], F32, tag="cs")
                rev_s = revb2_f if PSW else revb_f
                nc.vector.tensor_scalar(
                    out=cs, in0=col0(dI8s), scalar1=-1.0, scalar2=rev_s,
                    op0=OP.mult, op1=OP.add,
                )
                if PSW:
                    vp = col0(dV8p)
                    cp = sm.tile([P, 8], F32, tag="cp")
                    nc.vector.tensor_scalar(
                        out=cp, in0=col0(dI8p), scalar1=-1.0, scalar2=revb_f,
                        op0=OP.mult, op1=OP.add,
                    )
                    mA = sm.tile([P, 8], F32, tag="mA")
                    nc.vector.tensor_tensor(mA, vp, vs, op=OP.max)
                else:
                    mA = vs
                mAllD = sm.tile([P, 8], F32, tag="mAllD")
                nc.gpsimd.partition_all_reduce(mAllD, mA, channels=P,
                                               reduce_op=RED.max)
                eqs = sm.tile([P, 8], F32, tag="eqs")
                nc.vector.tensor_tensor(eqs, vs, mAllD, op=OP.is_equal)
                nc.vector.tensor_tensor(eqs, eqs, cs, op=OP.mult)
                if PSW:
                    eqp = sm.tile([P, 8], F32, tag="eqp")
                    nc.vector.tensor_tensor(eqp, vp, mAllD, op=OP.is_equal)
                    nc.vector.tensor_tensor(eqp, eqp, cp, op=OP.mult)
                    nc.vector.tensor_tensor(eqs, eqs, eqp, op=OP.max)
                wcD = sm.tile([P, 8], F32, tag="wcD")
                nc.gpsimd.partition_all_reduce(wcD, eqs, channels=P,
                                               reduce_op=RED.max)
                idxd = sm.tile([P, 8], F32, tag="idxd")
                nc.vector.tensor_scalar(
                    out=idxd, in0=wcD, scalar1=-1.0, scalar2=float(N),
                    op0=OP.mult, op1=OP.add,
                )
                nc.scalar.copy(out_i[0:1, BPC : 2 * BPC], idxd[0:1, :])

            # ---------- schedule ----------
            for b in range(BPC):
                y_reduce(b)
            y_finale(0)
            for b in range(GRP):
                d_fronty(b)
            y_finale(1)
            for b in range(GRP, BPC):
                d_fronty(b)
            for b in range(BPC):
                d_frontz(b)
                d_add1(b)
                d_sqx(b)
                d_tail(b)
            d_finale()
            nc.sync.dma_start(out[:, :], out_i[:, :])

    nc.compile()
    return nc


_NC_CACHE = None


def _get_nc():
    global _NC_CACHE
    if _NC_CACHE is None:
        _NC_CACHE = build_nc()
    return _NC_CACHE


def kernel(xyz: np.ndarray) -> np.ndarray:
    from concourse.bass_utils import run_bass_kernel_spmd

    assert xyz.shape == (1, B, 3, N), xyz.shape
    xyz = np.ascontiguousarray(xyz, dtype=np.float32)
    nc = _get_nc()
    in_maps = [
        {"xyz": np.ascontiguousarray(xyz[0, k * BPC : (k + 1) * BPC])}
        for k in range(N_CORES)
    ]
    res = run_bass_kernel_spmd(nc, in_maps, core_ids=list(range(N_CORES)))
    outs = [res.results[k]["idx"].reshape(2, BPC).T for k in range(N_CORES)]
    return np.concatenate(outs, axis=0).astype(np.int64)


# revision 4
# speedup vs baseline: 1.1204x; 1.1204x over previous
"""FPS (npoint=2) Bass kernel v5.

Per core (8 batches, planes [128,2048] fp32):
- DMA order staggers y early (centroid latency), x spread late (d-phase
  cadence). All HWDGE triggers issued first.
- y argmax: per-batch rowmax, partition_all_reduce finales (no PE
  transposes), ONE batched indirect row-gather per 4-batch group
  ([P,4,16]), tiny in-row localization, grouped centroid gather.
- d phase: Scalar squares (bias=-c), add1 on GpSimd, add2 split: cols
  [0,PSW) summed on PE into PSUM (identity fp32 matmuls), cols
  [PSW,2048) via DVE tensor_add in-place; MAX8+FIND per piece; one
  PAR-based finale at the end.
"""

import numpy as np

import concourse.bacc as bacc
import concourse.bass as bass
import concourse.bass_isa as bass_isa
import concourse.mybir as mybir
from concourse.masks import make_identity
from concourse.tile import TileContext

B = 64
N_CORES = 8
BPC = B // N_CORES
N = 262144
P = 128
COLS = N // P  # 2048
GRP = 4

F32 = mybir.dt.float32
U32 = mybir.dt.uint32
I32 = mybir.dt.int32
AX = mybir.AxisListType.X
OP = mybir.AluOpType
RED = bass_isa.ReduceOp
SQUARE = mybir.ActivationFunctionType.Square

# tuning knobs
PSW = 1536  # psum (PE-add) width: cols [0,PSW) summed on PE
YSPLIT = 8  # y-reds b < YSPLIT on DVE, rest GpSimd

SBW = COLS - PSW

DMA_ORDER = (
    [("y", b) for b in range(8)]
    + [p for b in range(8) for p in (("z", b), ("x", b))]
)
AS = 512  # add1 cols [0:AS] on DVE, rest GpSimd


def build_nc():
    nc = bacc.Bacc()
    xin = nc.dram_tensor("xyz", [BPC, 3, N], F32, kind="ExternalInput")
    out = nc.dram_tensor("idx", [1, 2 * BPC], I32, kind="ExternalOutput")
    xrows16 = xin.rearrange("b c n -> (b c n)").rearrange("(r k) -> r k", k=16)
    xflat = xin.rearrange("b c n -> (b c n)")[:, None]

    with TileContext(nc) as tc:
        with (
            tc.tile_pool(name="consts", bufs=1) as consts,
            tc.tile_pool(name="yp", bufs=BPC) as yp,
            tc.tile_pool(name="zp", bufs=BPC) as zp,
            tc.tile_pool(name="xp", bufs=BPC) as xp,
            tc.tile_pool(name="sm", bufs=2) as sm,
            tc.tile_pool(name="acc", bufs=1) as acc,
            tc.tile_pool(name="pd", bufs=2, space="PSUM") as pdp,
            tc.tile_pool(name="psS", bufs=1, space="PSUM") as psS,
        ):
            # ---- plane DMAs first (Sync engine streams them in order) ----
            tys, tzs, txs = [None] * BPC, [None] * BPC, [None] * BPC
            pools = {"y": (yp, tys, 1), "z": (zp, tzs, 2), "x": (xp, txs, 0)}
            for kind, b in DMA_ORDER:
                pool, store, c = pools[kind]
                t = pool.tile([P, COLS], F32, tag="t")
                store[b] = t
                nc.sync.dma_start(t, xin[b, c].rearrange("(p m) -> p m", p=P))

            # ---- constants ----
            ident = consts.tile([P, P], F32)
            make_identity(nc, ident)
            ones = consts.tile([1, P], F32)
            nc.vector.memset(ones, 1.0)

            def iota_f32(tag, shape, pattern, base, chan):
                ti = consts.tile(shape, I32, tag=f"{tag}_i")
                nc.gpsimd.iota(ti, pattern=pattern, base=base, channel_multiplier=chan)
                tf = consts.tile(shape, F32, tag=f"{tag}_f")
                nc.vector.tensor_copy(tf, ti)
                return tf

            # revb[p] = N - 2048 p ; revb2[p] = N - PSW - 2048 p
            revb_f = iota_f32("revb", [P, 1], [[0, 1]], N, -COLS)
            revb2_f = (
                iota_f32("revb2", [P, 1], [[0, 1]], N - PSW, -COLS) if PSW else None
            )
            # cand16[p, j] = 2048 - (16 p + j)
            cand16 = iota_f32("cand16", [P, 16], [[-1, 16]], COLS, -16)
            # candP8[p, b] = 128 - p
            candP8 = iota_f32("candP8", [P, 8], [[0, 8]], P, -1)
            # colbase8[p, b] = p + (b*3N + N)/16 + 128*128
            # (iota step limit is int16, so build as 16*(b*3072 + 2048) + p)
            iotaP_f = iota_f32("iotaP", [P, 1], [[0, 1]], 0, 1)
            cA = iota_f32("cA", [P, 8], [[3 * N // 256, 8]], (N // 16 + P * P) // 16, 0)
            colbase8 = consts.tile([P, 8], F32, tag="colbase8")
            nc.vector.tensor_scalar(
                out=colbase8, in0=cA, scalar1=16.0, scalar2=iotaP_f,
                op0=OP.mult, op1=OP.add,
            )
            # bpn12[q] = N * q  (q = 3*(b-lo) + c)
            bpn12 = iota_f32("bpn12", [3 * GRP, 1], [[0, 1]], 0, N)
            # selmask[q, b] = 1 iff q//3 == b, for q in 0..11, b in 0..3:
            #   (q >= 3b) && (q <= 3b + 2)
            qv = iota_f32("qv", [3 * GRP, GRP], [[0, GRP]], 0, 1)
            c3 = iota_f32("c3", [3 * GRP, GRP], [[3, GRP]], 0, 0)
            m1 = consts.tile([3 * GRP, GRP], F32, tag="m1")
            nc.vector.tensor_tensor(m1, qv, c3, op=OP.is_ge)
            m2 = consts.tile([3 * GRP, GRP], F32, tag="m2")
            nc.vector.tensor_scalar(
                out=m2, in0=c3, scalar1=2.0, scalar2=None, op0=OP.add
            )
            selmask = consts.tile([3 * GRP, GRP], F32, tag="selmask")
            nc.vector.tensor_tensor(m2, qv, m2, op=OP.is_le)
            nc.vector.tensor_tensor(selmask, m1, m2, op=OP.mult)

            out_i = acc.tile([1, 2 * BPC], I32)
            dY = acc.tile([P, 8], F32)
            mxAll = acc.tile([P, 8], F32)
            wcAll = acc.tile([P, 8], F32)
            wr = acc.tile([P, 8], F32)
            wjAll = acc.tile([P, 8], F32)
            idxAll = acc.tile([P, 8], F32)
            offs8u = acc.tile([P, 8], U32)
            growA = acc.tile([P, 2 * GRP * 16], F32)
            negcY = acc.tile([P, 8], F32)
            negc_g = [None, None]
            dV8p = acc.tile([P, 8 * BPC], F32)
            dI8p = acc.tile([P, 8 * BPC], U32)
            dV8s = acc.tile([P, 8 * BPC], F32)
            dI8s = acc.tile([P, 8 * BPC], U32)

            def col0(t, k=8):
                return t.rearrange("p (b k) -> p b k", k=k)[:, :, 0]

            # ---------- y phase ----------
            def y_reduce(b):
                eng = nc.vector if b < YSPLIT else nc.gpsimd
                eng.tensor_reduce(dY[:, b : b + 1], tys[b], axis=AX, op=OP.max)

            def y_finale(g):
                lo = g * GRP
                sl = slice(lo, lo + GRP)
                nc.gpsimd.partition_all_reduce(
                    mxAll[:, sl], dY[:, sl], channels=P, reduce_op=RED.max
                )
                nc.vector.tensor_scalar(
                    out=negcY[:, sl], in0=mxAll[:, sl], scalar1=-1.0,
                    scalar2=None, op0=OP.mult,
                )
                eq = sm.tile([P, GRP], F32, tag=f"eq{g}")
                nc.vector.tensor_tensor(eq, dY[:, sl], mxAll[:, sl], op=OP.is_equal)
                nc.vector.tensor_tensor(eq, eq, candP8[:, 0:GRP], op=OP.mult)
                nc.gpsimd.partition_all_reduce(
                    wcAll[:, sl], eq, channels=P, reduce_op=RED.max
                )
                # row-gather offsets: colbase8[:, b] - 128*wc
                offs_f = sm.tile([P, GRP], F32, tag=f"offs{g}")
                for b in range(lo, lo + GRP):
                    nc.vector.tensor_scalar(
                        out=offs_f[:, b - lo : b - lo + 1],
                        in0=wcAll[:, b : b + 1],
                        scalar1=-float(P), scalar2=colbase8[:, b : b + 1],
                        op0=OP.mult, op1=OP.add,
                    )
                nc.vector.tensor_copy(offs8u[:, sl], offs_f)
                for b in range(lo, lo + GRP):
                    nc.gpsimd.indirect_dma_start(
                        out=growA[:, b * 16 : (b + 1) * 16], out_offset=None,
                        in_=xrows16,
                        in_offset=bass.IndirectOffsetOnAxis(
                            ap=offs8u[:, b : b + 1], axis=0
                        ),
                    )
                for b in range(lo, lo + GRP):
                    c2 = sm.tile([P, 16], F32, tag=f"c2{g}")
                    nc.vector.scalar_tensor_tensor(
                        out=c2, in0=growA[:, b * 16 : b * 16 + 16],
                        scalar=mxAll[:, b : b + 1], in1=cand16,
                        op0=OP.is_equal, op1=OP.mult,
                    )
                    nc.vector.tensor_reduce(wr[:, b : b + 1], c2, axis=AX, op=OP.max)
                nc.gpsimd.partition_all_reduce(
                    wjAll[:, sl], wr[:, sl], channels=P, reduce_op=RED.max
                )
                # idx = 264192 - 2048*wc - wj
                tmp = sm.tile([P, GRP], F32, tag=f"tmp{g}")
                nc.vector.tensor_scalar(
                    out=tmp, in0=wjAll[:, sl], scalar1=-1.0,
                    scalar2=float(COLS * (P + 1)), op0=OP.mult, op1=OP.add,
                )
                wt = sm.tile([P, GRP], F32, tag=f"wt{g}")
                nc.vector.tensor_scalar(
                    out=wt, in0=wcAll[:, sl], scalar1=-float(COLS), scalar2=None,
                    op0=OP.mult,
                )
                nc.vector.tensor_tensor(idxAll[:, sl], wt, tmp, op=OP.add)
                nc.scalar.copy(out_i[0:1, lo : lo + GRP], idxAll[0:1, sl])
                # centroid gather: sel12[q] = idxAll[q, lo + q//3]
                selv = sm.tile([3 * GRP, GRP], F32, tag=f"selv{g}")
                nc.vector.tensor_tensor(
                    selv, idxAll[0 : 3 * GRP, sl], selmask, op=OP.mult
                )
                sel12 = sm.tile([3 * GRP, 1], F32, tag=f"sel12{g}")
                nc.vector.tensor_reduce(sel12, selv, axis=AX, op=OP.max)
                offs12f = sm.tile([3 * GRP, 1], F32, tag=f"offs12f{g}")
                nc.vector.tensor_scalar(
                    out=offs12f, in0=sel12, scalar1=float(lo * 3 * N),
                    scalar2=bpn12, op0=OP.add, op1=OP.add,
                )
                offs12 = sm.tile([3 * GRP, 1], U32, tag=f"offs12u{g}")
                nc.vector.tensor_copy(offs12, offs12f)
                cg = sm.tile([3 * GRP, 1], F32, tag=f"cg{g}")
                nc.gpsimd.indirect_dma_start(
                    out=cg, out_offset=None, in_=xflat,
                    in_offset=bass.IndirectOffsetOnAxis(ap=offs12, axis=0),
                )
                pcr = psS.tile([1, 3 * GRP], F32, tag="pcr")
                nc.tensor.transpose(pcr, cg, ident[0 : 3 * GRP, 0 : 3 * GRP])
                negrow = sm.tile([1, 3 * GRP], F32, tag=f"negrow{g}")
                nc.scalar.mul(negrow, pcr, -1.0)
                pneg = psS.tile([P, 3 * GRP], F32, tag="pneg")
                nc.tensor.matmul(pneg, ones, negrow, start=True, stop=True)
                negc = sm.tile([P, 3 * GRP], F32, tag=f"negc{g}")
                nc.scalar.copy(negc, pneg)
                negc_g[g] = negc

            def nbias(b, c):
                if c == 1:
                    return negcY[:, b : b + 1]
                g = b // GRP
                j = 3 * (b - GRP * g) + c
                return negc_g[g][:, j : j + 1]

            # ---------- d phase (split into emission stages) ----------
            def d_fronty(b):
                nc.scalar.activation(tys[b], tys[b], SQUARE, bias=nbias(b, 1))

            def d_frontz(b):
                nc.scalar.activation(tzs[b], tzs[b], SQUARE, bias=nbias(b, 2))

            def d_add1(b):
                ty, tz = tys[b], tzs[b]
                if AS:
                    nc.vector.tensor_add(tz[:, 0:AS], ty[:, 0:AS], tz[:, 0:AS])
                if AS < COLS:
                    nc.gpsimd.tensor_add(tz[:, AS:], ty[:, AS:], tz[:, AS:])

            def d_sqx(b):
                nc.scalar.activation(txs[b], txs[b], SQUARE, bias=nbias(b, 0))

            pd_list = [None] * BPC

            def d_pe(b):
                tz, tx = tzs[b], txs[b]
                if PSW:
                    pd = pdp.tile([P, PSW], F32, tag="pd")
                    pd_list[b] = pd
                    for ch in range(PSW // 512):
                        slc = slice(512 * ch, 512 * (ch + 1))
                        nc.tensor.matmul(pd[:, slc], ident, tz[:, slc],
                                         start=True, stop=False)
                        nc.tensor.matmul(pd[:, slc], ident, tx[:, slc],
                                         start=False, stop=True)
                sb = slice(PSW, COLS)
                nc.gpsimd.tensor_add(tx[:, sb], tz[:, sb], tx[:, sb])

            def d_scan(b):
                tx = txs[b]
                sb = slice(PSW, COLS)
                nc.vector.max(out=dV8s[:, 8 * b : 8 * b + 8], in_=tx[:, sb])
                nc.vector.max_index(dI8s[:, 8 * b : 8 * b + 8],
                                    dV8s[:, 8 * b : 8 * b + 8], tx[:, sb])
                if PSW:
                    pd = pd_list[b]
                    nc.vector.max(out=dV8p[:, 8 * b : 8 * b + 8], in_=pd)
                    nc.vector.max_index(dI8p[:, 8 * b : 8 * b + 8],
                                        dV8p[:, 8 * b : 8 * b + 8], pd)

            def d_finale():
                vs = col0(dV8s)
                cs = sm.tile([P, 8], F32, tag="cs")
                rev_s = revb2_f if PSW else revb_f
                nc.vector.tensor_scalar(
                    out=cs, in0=col0(dI8s), scalar1=-1.0, scalar2=rev_s,
                    op0=OP.mult, op1=OP.add,
                )
                if PSW:
                    vp = col0(dV8p)
                    cp = sm.tile([P, 8], F32, tag="cp")
                    nc.vector.tensor_scalar(
                        out=cp, in0=col0(dI8p), scalar1=-1.0, scalar2=revb_f,
                        op0=OP.mult, op1=OP.add,
                    )
                    mA = sm.tile([P, 8], F32, tag="mA")
                    nc.vector.tensor_tensor(mA, vp, vs, op=OP.max)
                else:
                    mA = vs
                mAllD = sm.tile([P, 8], F32, tag="mAllD")
                nc.gpsimd.partition_all_reduce(mAllD, mA, channels=P,
                                               reduce_op=RED.max)
                eqs = sm.tile([P, 8], F32, tag="eqs")
                nc.vector.tensor_tensor(eqs, vs, mAllD, op=OP.is_equal)
                nc.vector.tensor_tensor(eqs, eqs, cs, op=OP.mult)
                if PSW:
                    eqp = sm.tile([P, 8], F32, tag="eqp")
                    nc.vector.tensor_tensor(eqp, vp, mAllD, op=OP.is_equal)
                    nc.vector.tensor_tensor(eqp, eqp, cp, op=OP.mult)
                    nc.vector.tensor_tensor(eqs, eqs, eqp, op=OP.max)
                wcD = sm.tile([P, 8], F32, tag="wcD")
                nc.gpsimd.partition_all_reduce(wcD, eqs, channels=P,
                                               reduce_op=RED.max)
                idxd = sm.tile([P, 8], F32, tag="idxd")
                nc.vector.tensor_scalar(
                    out=idxd, in0=wcD, scalar1=-1.0, scalar2=float(N),
                    op0=OP.mult, op1=OP.add,
                )
                nc.scalar.copy(out_i[0:1, BPC : 2 * BPC], idxd[0:1, :])

            # ---------- schedule ----------
            for b in range(BPC):
                y_reduce(b)
            y_finale(0)
            for b in range(GRP):
                d_fronty(b)
            y_finale(1)
            for b in range(GRP, BPC):
                d_fronty(b)
            for b in range(BPC):
                d_frontz(b)
                d_add1(b)
                d_sqx(b)
                d_pe(b)
                if b >= 1:
                    d_scan(b - 1)
            d_scan(BPC - 1)
            d_finale()
            nc.sync.dma_start(out[:, :], out_i[:, :])

    nc.compile()
    return nc


_NC_CACHE = None


def _get_nc():
    global _NC_CACHE
    if _NC_CACHE is None:
        _NC_CACHE = build_nc()
    return _NC_CACHE


def kernel(xyz: np.ndarray) -> np.ndarray:
    from concourse.bass_utils import run_bass_kernel_spmd

    assert xyz.shape == (1, B, 3, N), xyz.shape
    xyz = np.ascontiguousarray(xyz, dtype=np.float32)
    nc = _get_nc()
    in_maps = [
        {"xyz": np.ascontiguousarray(xyz[0, k * BPC : (k + 1) * BPC])}
        for k in range(N_CORES)
    ]
    res = run_bass_kernel_spmd(nc, in_maps, core_ids=list(range(N_CORES)))
    outs = [res.results[k]["idx"].reshape(2, BPC).T for k in range(N_CORES)]
    return np.concatenate(outs, axis=0).astype(np.int64)
